# revision 63
# baseline (speedup 1.0000x reference)
"""Multi-head attention (B=2, S=4096, D=512, H=8) on 8 Trainium2 NeuronCores.

Sharding: core c handles batch b = c // 4 and head-group g = c % 4 (2 heads =
columns/rows [128g : 128g+128] of the projection weights).  Each core runs its
2 heads' attention over the full sequence plus the partial output projection
through the matching 128 rows of Wo; the host sums the 4 partials per batch
and adds bo (pure unshard for row-parallel Wo).

Numerics: fp16 storage for X/W/q/k/v/P/ctx, fp32 PSUM accumulation, fp32
softmax denominators, fp16 output partials (summed fp32 host-side).

v2 pipeline (vs baseline):
  - DMA-transposes ordered k.h0, q.h0, k.h1, v.h0, v.h1, q.h1 so QK/exp can
    start ~25us in; k is fully resident right when iter 0 needs its 2nd half.
  - PV runs one full iteration behind QK/exp (P tiles buffered in SBUF), so
    early PVs never block the PE queue waiting on v transposes.
  - softmax normalization via rank-1 broadcast: the denominator row of
    [uctx.T | denom] is reciprocated as a row (DVE), broadcast to a [128,512]
    PSUM tile with a 1-row fp16 matmul, and multiplied into uctx during the
    fp16 downcast (DVE).  No PE tail transposes, no ACT copies.
  - output projection is ONE matmul per s-tile (both heads contract together
    since uctx16 rows 0:64 / 64:128 are the two heads), fp16 out partials.
"""

import os

import numpy as np

import concourse.bass as bass
import concourse.tile as tile
from concourse import bacc, mybir
from concourse.bass_utils import run_bass_kernel_spmd
from concourse.masks import make_identity

P = 128
D = 512
GD = 128  # head-group width: 2 heads x 64
HD = 64
S_FULL = 4096
B_FULL = 2
N_CORES = 8
F32 = mybir.dt.float32
F16 = mybir.dt.float16
EXP = mybir.ActivationFunctionType.Exp


def _emit(tc, S, io):
    nc = tc.nc
    NT = S // P  # 128-wide s/k tiles
    SB = S // 512  # 512-wide s blocks
    QB = S // 512  # query blocks
    CH = 3  # key-tiles per exp chunk (3 PSUM banks, x2 buffered)

    xq, xk, xv, wq, wk, wv, wo, bq, bk, bv, bo, out = io

    with (
        tc.tile_pool(name="persist", bufs=1) as pp,
        tc.tile_pool(name="lgp", bufs=2, space="PSUM") as lgp,
        tc.tile_pool(name="mpsum", bufs=1, space="PSUM") as mp,
        tc.tile_pool(name="pbp", bufs=1, space="PSUM") as pbp,
        tc.tile_pool(name="xtp", bufs=8) as xtp,
        tc.tile_pool(name="vstage", bufs=4) as vsp,
        tc.tile_pool(name="ptp", bufs=25) as ptp,
        tc.tile_pool(name="ucp", bufs=3) as ucp,
        tc.tile_pool(name="obp", bufs=4) as obp,
    ):
        ident16 = pp.tile([P, P], F16, name="ident16")
        make_identity(nc, ident16)

        # fp16 weights (pre-cast AND pre-packed on host) — on the sync queue
        # BEFORE the DMA-transposes: concurrent non-transpose DMA traffic
        # interleaves exclusively with the XBAR and opens ~8.5us gaps between
        # transposes, so fewer DMAs here = earlier transpose start.
        # wq holds [Wk; Wq; Wv] stacked (k first: its projection is needed
        # soonest); bq holds [bk | bq | bv] columns.
        wall = pp.tile([P, 12, GD], F16, name="wall")
        nc.sync.dma_start(wall, wq.rearrange("(w t p) m -> p (w t) m", p=P, t=4))
        wks = wall[:, 0:4, :]
        wqs = wall[:, 4:8, :]
        wvs = wall[:, 8:12, :]
        wos = pp.tile([P, D], F16, name="wos")
        nc.sync.dma_start(wos, wo)
        ball = pp.tile([P, 3], F32, name="ball")
        nc.sync.dma_start(ball, bq.rearrange("(w p) -> p w", p=P))
        bks = ball[:, 0:1]
        bqs = ball[:, 1:2]
        bvs = ball[:, 2:3]

        # big persistent activations (all fp16).  Partition-offset memsets
        # stay on the DVE (proven on HW); gpsimd only does full-partition
        # memsets (no DMA-fabric use either way).
        kT = pp.tile([P, S], F16, name="kT")
        # q per-head, zero-padded to 128 partitions so QK contracts over
        # K=128 (K=64 matmuls stream at ~half rate on the PE)
        qT0 = pp.tile([P, S], F16, name="qT0")
        qT1 = pp.tile([P, S], F16, name="qT1")
        qTh = [qT0, qT1]
        nc.vector.memset(qT0[HD:P, :], 0.0)
        nc.vector.memset(qT1[0:HD, :], 0.0)
        # v in natural [keys, hd] layout per k-tile, per head; head0's v at
        # free cols 0:64 with a ones-column at 64, head1's v at cols 64:128
        # with its ones-column at 0 — so PV output rows line up with the
        # head's rows in uctx16 and the denominator row is 64 / 0.
        vaug = pp.tile([P, 2, NT, P], F16, name="vaug")
        nc.gpsimd.memset(vaug, 0.0)
        nc.gpsimd.memset(vaug[:, 0, :, HD : HD + 1], 1.0)
        nc.gpsimd.memset(vaug[:, 1, :, 0:1], 1.0)
        # unnormalized ctx.T, one zero-padded tile per head (head h's rows
        # live at 64h:64h+64, the other half stays zero) so the phase C
        # matmuls contract over K=128 at full PE rate.
        uctxA = pp.tile([P, S], F16, name="uctxA")
        uctxB = pp.tile([P, S], F16, name="uctxB")
        uctxh = [uctxA, uctxB]
        nc.vector.memset(uctxA[HD:P, :], 0.0)
        nc.vector.memset(uctxB[0:HD, :], 0.0)
        # per-(head, s-tile) reciprocal softmax denominators as
        # per-partition columns
        rd = pp.tile([P, 2, NT], F32, name="rd")

        # ---------------- Phase A ------------------------------------------
        # All three inputs go through the DMA-transpose XBAR (sync HWDGE
        # queue, the serial resource) in variable-size row-range units,
        # ordered by when phase B first consumes them: k.h0, then just q's
        # first 512-row block (so exp starts ~17us in), k.h1, v (PV is a full
        # iteration behind), then the rest of q.
        SH = S // 2
        xt_tiles = {}  # (which, lo) -> list of 4 xt tiles

        def emit_transposes(which, lo, nrows):
            src = {"k": xk, "v": xv, "q": xq}[which]
            xts = []
            for dt_ in range(4):
                xt = xtp.tile([P, SH], F16, tag="xt", name="xt")[:, :nrows]
                nc.sync.dma_start(
                    xt,
                    src[lo : lo + nrows, dt_ * P : (dt_ + 1) * P],
                    transpose=True,
                )
                xts.append(xt)
            xt_tiles[(which, lo)] = xts

        def emit_proj(which, lo, nrows):
            w = {"k": wks, "v": wvs, "q": wqs}[which]
            xts = xt_tiles[(which, lo)]
            for sbl in range(nrows // 512):
                sb = lo // 512 + sbl
                cols = slice(sb * 512, (sb + 1) * 512)
                lcol = slice(sbl * 512, (sbl + 1) * 512)
                acc = mp.tile([P, 512], F32, tag="mA", name="acc")
                for dt_ in range(4):
                    nc.tensor.matmul(
                        acc,
                        lhsT=w[:, dt_, :],
                        rhs=xts[dt_][:, lcol],
                        start=(dt_ == 0),
                        stop=(dt_ == 3),
                    )
                if which == "q":
                    nc.vector.tensor_scalar_add(
                        qT0[0:HD, cols], acc[0:HD, :], bqs[0:HD, :]
                    )
                    nc.vector.tensor_scalar_add(
                        qT1[HD:P, cols], acc[HD:P, :], bqs[HD:P, :]
                    )
                elif which == "k":
                    nc.vector.tensor_scalar_add(kT[:, cols], acc[:], bks[:])
                else:
                    vt = vsp.tile([P, 512], F16, tag="vt", name="vt")
                    nc.vector.tensor_scalar_add(vt, acc[:], bvs[:])
                    for j in range(4):
                        kt_i = sb * 4 + j
                        ps2 = mp.tile([P, P], F16, tag="mA", name="ps2")
                        nc.tensor.transpose(
                            ps2, vt[:, j * P : (j + 1) * P], ident16
                        )
                        nc.vector.tensor_copy(
                            out=vaug[:, 0, kt_i, 0:HD], in_=ps2[:, 0:HD]
                        )
                        nc.vector.tensor_copy(
                            out=vaug[:, 1, kt_i, HD:P], in_=ps2[:, HD:P]
                        )

        # ------- Phase B: attention, PV lagged one iteration ----------------
        iters = [(qb, h) for qb in range(QB) for h in (0, 1)]
        CPI = (NT + CH - 1) // CH  # chunks per iteration
        chunks = [
            (it_idx, c0)
            for it_idx in range(len(iters))
            for c0 in range(0, NT, CH)
        ]
        LAG = CPI  # PV trails QK/exp by one full iteration
        lg_tiles = {}
        pt_tiles = {}
        pv_tiles = {}

        def emit_qk(j):
            it_idx, c0 = chunks[j]
            qb, h = iters[it_idx]
            qcols = slice(qb * 512, (qb + 1) * 512)
            n = min(CH, NT - c0)
            lg = lgp.tile([P, CH * 512], F32, tag="lg", name="lg")
            for i in range(n):
                kt_i = c0 + i
                nc.tensor.matmul(
                    lg[:, i * 512 : (i + 1) * 512],
                    lhsT=kT[:, kt_i * P : (kt_i + 1) * P],
                    rhs=qTh[h][:, qcols],
                    start=True,
                    stop=True,
                )
            lg_tiles[j] = lg

        def emit_exp(j):
            it_idx, c0 = chunks[j]
            n = min(CH, NT - c0)
            lg = lg_tiles.pop(j)
            ptt = ptp.tile([P, CH * 512], F16, tag="pt", name="ptt")
            nc.scalar.activation(
                ptt[:, : n * 512], lg[:, : n * 512], EXP, scale=0.125
            )
            pt_tiles[j] = ptt

        def emit_pv(j):
            it_idx, c0 = chunks[j]
            qb, h = iters[it_idx]
            n = min(CH, NT - c0)
            ptt = pt_tiles.pop(j)
            if it_idx not in pv_tiles:
                pv_tiles[it_idx] = pbp.tile(
                    [P, 512], F32, tag="pb", name="pv_acc"
                )
            pv_acc = pv_tiles[it_idx]
            for i in range(n):
                kt_i = c0 + i
                nc.tensor.matmul(
                    pv_acc,
                    lhsT=vaug[:, h, kt_i, :],
                    rhs=ptt[:, i * 512 : (i + 1) * 512],
                    start=(kt_i == 0),
                    stop=(kt_i == NT - 1),
                )
            if c0 + CH >= NT:
                finish_iter(it_idx)

        def finish_iter(it_idx):
            # [uctx.T | denom] -> SBUF; denominator row -> per-partition
            # columns via PE transposes; [128,1] column reciprocals (cheap
            # multi-partition DVE).  Normalization happens in phase C.
            qb, h = iters[it_idx]
            qcols = slice(qb * 512, (qb + 1) * 512)
            rows = slice(h * HD, (h + 1) * HD)
            pv_acc = pv_tiles.pop(it_idx)
            dr = HD if h == 0 else 0  # denominator row in the PV output
            # fp16 staging: halves the copy, and the tail transposes run at
            # 1 cycle/row instead of fp32's 2
            uc = ucp.tile([P, 512], F16, tag="uc", name="uc")
            nc.vector.tensor_copy(out=uc, in_=pv_acc[:])
            nc.vector.tensor_copy(out=uctxh[h][rows, qcols], in_=uc[rows, :])
            for sl in range(4):
                st = 4 * qb + sl
                tps = mp.tile([P, P], F16, tag="mA", name="tps")
                nc.tensor.transpose(
                    tps, uc[:, sl * P : (sl + 1) * P], ident16
                )
                nc.vector.reciprocal(
                    rd[:, h, st : st + 1], tps[:, dr : dr + 1]
                )

        # phase C in two half-steps per s-tile (single mp PSUM bank, no
        # PE-waits-on-DVE): A: ops0 = uctxA.T @ Wo (K=128, h0 rows live);
        # ob0 = ops0 * rd0.  B: ops1 = uctxB.T @ Wo; ob = ops1*rd1 + ob0; DMA.
        ob0_tiles = {}

        def emit_phase_c_half(st, phase):
            stcols = slice(st * P, (st + 1) * P)
            if phase == 0:
                ops0 = mp.tile([P, D], F32, tag="mA", name="ops0")
                nc.tensor.matmul(
                    ops0, lhsT=uctxA[:, stcols], rhs=wos, start=True, stop=True
                )
                ob0 = obp.tile([P, D], F32, tag="ob0", name="ob0")
                nc.vector.tensor_scalar_mul(ob0, ops0[:], rd[:, 0, st : st + 1])
                ob0_tiles[st] = ob0
            else:
                ops1 = mp.tile([P, D], F32, tag="mA", name="ops1")
                nc.tensor.matmul(
                    ops1, lhsT=uctxB[:, stcols], rhs=wos, start=True, stop=True
                )
                ob0 = ob0_tiles.pop(st)
                ob = obp.tile([P, D], F16, tag="ob", name="ob")
                nc.vector.scalar_tensor_tensor(
                    out=ob,
                    in0=ops1[:],
                    scalar=rd[:, 1, st : st + 1],
                    in1=ob0[:],
                    op0=mybir.AluOpType.mult,
                    op1=mybir.AluOpType.add,
                )
                nc.sync.dma_start(out[st * P : (st + 1) * P, :], ob)

        # ---- emission schedule ----
        # XBAR units (sync queue, serial), in first-consumption order;
        # projections emitted in the same order so the single mp PSUM bank's
        # rotation matches execution.
        units = (
            ("k", 0, SH),
            ("q", 0, 512),
            ("k", SH, SH),
            ("v", 0, SH),
            ("v", SH, SH),
            ("q", 512, 512),
            ("q", 1024, 1024),
            ("q", 2048, 2048),
        )
        for u in units:
            emit_transposes(*u)
        for u in units:
            emit_proj(*u)

        n_chunks = len(chunks)
        pc_queue = []  # pending phase C half-steps, drained 1 per loop step
        emit_qk(0)
        for j in range(n_chunks + LAG):
            if j < n_chunks:
                emit_exp(j)
                if j + 1 < n_chunks:
                    emit_qk(j + 1)
            if j >= LAG:
                emit_pv(j - LAG)
                it_idx, c0 = chunks[j - LAG]
                if c0 + CH >= NT and it_idx % 2 == 1:
                    qb = it_idx // 2
                    for sl in range(4):
                        pc_queue.append((4 * qb + sl, 0))
                        pc_queue.append((4 * qb + sl, 1))
            if pc_queue:
                emit_phase_c_half(*pc_queue.pop(0))
        while pc_queue:
            emit_phase_c_half(*pc_queue.pop(0))


def build(S=S_FULL, enable_asserts=False):
    nc = bacc.Bacc(
        "TRN2",
        target_bir_lowering=False,
        debug=False,
        enable_asserts=enable_asserts,
        num_devices=N_CORES,
    )
    xq = nc.dram_tensor("xq", [S, D], F16, kind="ExternalInput").ap()
    xk = nc.dram_tensor("xk", [S, D], F16, kind="ExternalInput").ap()
    xv = nc.dram_tensor("xv", [S, D], F16, kind="ExternalInput").ap()
    # packed [Wk; Wq; Wv] and [bk | bq | bv] (fewer pre-transpose DMAs)
    wq = nc.dram_tensor("wpack", [3 * D, GD], F16, kind="ExternalInput").ap()
    wo = nc.dram_tensor("wo", [GD, D], F16, kind="ExternalInput").ap()
    bq = nc.dram_tensor("bpack", [3 * GD], F32, kind="ExternalInput").ap()
    out = nc.dram_tensor("out", [S, D], F16, kind="ExternalOutput").ap()
    io = (xq, xk, xv, wq, None, None, wo, bq, None, None, None, out)
    with tile.TileContext(nc) as tc:
        _emit(tc, S, io)
    nc.compile()
    return nc


def make_in_maps(queries, keys, values, Wq, bq, Wk, bk, Wv, bv, Wo, bo):
    f16 = lambda a: np.ascontiguousarray(np.asarray(a, dtype=np.float32).astype(np.float16))
    f32 = lambda a: np.ascontiguousarray(np.asarray(a, dtype=np.float32))
    in_maps = []
    for c in range(N_CORES):
        b, g = divmod(c, 4)
        sl = slice(g * GD, (g + 1) * GD)
        in_maps.append(
            {
                "xq": f16(queries[b]),
                "xk": f16(keys[b]),
                "xv": f16(values[b]),
                "wpack": f16(
                    np.concatenate(
                        [
                            np.asarray(Wk)[:, sl],
                            np.asarray(Wq)[:, sl],
                            np.asarray(Wv)[:, sl],
                        ],
                        axis=0,
                    )
                ),
                "wo": f16(np.asarray(Wo)[sl, :]),
                "bpack": f32(
                    np.concatenate(
                        [
                            np.asarray(bk)[sl],
                            np.asarray(bq)[sl],
                            np.asarray(bv)[sl],
                        ]
                    )
                ),
            }
        )
    return in_maps


_NC = None
last_results = None


def kernel(queries, keys, values, Wq, bq, Wk, bk, Wv, bv, Wo, bo):
    global _NC, last_results
    if _NC is None:
        _NC = build(S_FULL)
    in_maps = make_in_maps(
        queries, keys, values, Wq, bq, Wk, bk, Wv, bv, Wo, bo
    )
    res = run_bass_kernel_spmd(
        _NC,
        in_maps,
        core_ids=list(range(N_CORES)),
        trace=bool(int(os.environ.get("MHA_TRACE", "0"))),
    )
    last_results = res
    outs = [np.asarray(res.results[c]["out"], dtype=np.float32) for c in range(N_CORES)]
    full = np.empty((B_FULL, S_FULL, D), dtype=np.float32)
    bo32 = np.asarray(bo, dtype=np.float32)
    for b in range(B_FULL):
        full[b] = outs[4 * b] + outs[4 * b + 1] + outs[4 * b + 2] + outs[4 * b + 3]
        full[b] += bo32
    return full


# revision 64
# speedup vs baseline: 1.0124x; 1.0124x over previous
"""Multi-head attention (B=2, S=4096, D=512, H=8) on 8 Trainium2 NeuronCores.

Sharding: core c handles batch b = c // 4 and head-group g = c % 4 (2 heads =
columns/rows [128g : 128g+128] of the projection weights).  Each core runs its
2 heads' attention over the full sequence plus the partial output projection
through the matching 128 rows of Wo; the host sums the 4 partials per batch
and adds bo (pure unshard for row-parallel Wo).

Numerics: fp16 storage for X/W/q/k/v/P/ctx, fp32 PSUM accumulation, fp32
softmax denominators, fp16 output partials (summed fp32 host-side).

v2 pipeline (vs baseline):
  - DMA-transposes ordered k.h0, q.h0, k.h1, v.h0, v.h1, q.h1 so QK/exp can
    start ~25us in; k is fully resident right when iter 0 needs its 2nd half.
  - PV runs one full iteration behind QK/exp (P tiles buffered in SBUF), so
    early PVs never block the PE queue waiting on v transposes.
  - softmax normalization via rank-1 broadcast: the denominator row of
    [uctx.T | denom] is reciprocated as a row (DVE), broadcast to a [128,512]
    PSUM tile with a 1-row fp16 matmul, and multiplied into uctx during the
    fp16 downcast (DVE).  No PE tail transposes, no ACT copies.
  - output projection is ONE matmul per s-tile (both heads contract together
    since uctx16 rows 0:64 / 64:128 are the two heads), fp16 out partials.
"""

import os

import numpy as np

import concourse.bass as bass
import concourse.tile as tile
from concourse import bacc, mybir
from concourse.bass_utils import run_bass_kernel_spmd
from concourse.masks import make_identity

P = 128
D = 512
GD = 128  # head-group width: 2 heads x 64
HD = 64
S_FULL = 4096
B_FULL = 2
N_CORES = 8
F32 = mybir.dt.float32
F16 = mybir.dt.float16
EXP = mybir.ActivationFunctionType.Exp


def _emit(tc, S, io):
    nc = tc.nc
    NT = S // P  # 128-wide s/k tiles
    SB = S // 512  # 512-wide s blocks
    QB = S // 512  # query blocks
    CH = 3  # key-tiles per exp chunk (3 PSUM banks, x2 buffered)

    xq, xk, xv, wq, wk, wv, wo, bq, bk, bv, bo, out = io

    with (
        tc.tile_pool(name="persist", bufs=1) as pp,
        tc.tile_pool(name="lgp", bufs=2, space="PSUM") as lgp,
        tc.tile_pool(name="mpsum", bufs=1, space="PSUM") as mp,
        tc.tile_pool(name="pbp", bufs=1, space="PSUM") as pbp,
        tc.tile_pool(name="xtp", bufs=16) as xtp,
        tc.tile_pool(name="vstage", bufs=4) as vsp,
        tc.tile_pool(name="ptp", bufs=14) as ptp,
        tc.tile_pool(name="ucp", bufs=3) as ucp,
        tc.tile_pool(name="obp", bufs=4) as obp,
    ):
        ident16 = pp.tile([P, P], F16, name="ident16")
        make_identity(nc, ident16)

        # fp16 weights (pre-cast AND pre-packed on host) — on the sync queue
        # BEFORE the DMA-transposes: concurrent non-transpose DMA traffic
        # interleaves exclusively with the XBAR and opens ~8.5us gaps between
        # transposes, so fewer DMAs here = earlier transpose start.
        # wq holds [Wk; Wq; Wv] stacked (k first: its projection is needed
        # soonest); bq holds [bk | bq | bv] columns.
        wall = pp.tile([P, 12, GD], F16, name="wall")
        nc.sync.dma_start(wall, wq.rearrange("(w t p) m -> p (w t) m", p=P, t=4))
        wks = wall[:, 0:4, :]
        wqs = wall[:, 4:8, :]
        wvs = wall[:, 8:12, :]
        wos = pp.tile([P, D], F16, name="wos")
        nc.sync.dma_start(wos, wo)
        ball = pp.tile([P, 3], F32, name="ball")
        nc.sync.dma_start(ball, bq.rearrange("(w p) -> p w", p=P))
        bks = ball[:, 0:1]
        bqs = ball[:, 1:2]
        bvs = ball[:, 2:3]

        # big persistent activations (all fp16).  Partition-offset memsets
        # stay on the DVE (proven on HW); gpsimd only does full-partition
        # memsets (no DMA-fabric use either way).
        kT = pp.tile([P, S], F16, name="kT")
        # q per-head, zero-padded to 128 partitions so QK contracts over
        # K=128 (K=64 matmuls stream at ~half rate on the PE)
        qT0 = pp.tile([P, S], F16, name="qT0")
        qT1 = pp.tile([P, S], F16, name="qT1")
        qTh = [qT0, qT1]
        nc.vector.memset(qT0[HD:P, :], 0.0)
        nc.vector.memset(qT1[0:HD, :], 0.0)
        # v in natural [keys, hd] layout per k-tile, per head; head0's v at
        # free cols 0:64 with a ones-column at 64, head1's v at cols 64:128
        # with its ones-column at 0 — so PV output rows line up with the
        # head's rows in uctx16 and the denominator row is 64 / 0.
        vaug = pp.tile([P, 2, NT, P], F16, name="vaug")
        nc.gpsimd.memset(vaug, 0.0)
        nc.gpsimd.memset(vaug[:, 0, :, HD : HD + 1], 1.0)
        nc.gpsimd.memset(vaug[:, 1, :, 0:1], 1.0)
        # unnormalized ctx.T, one zero-padded tile per head (head h's rows
        # live at 64h:64h+64, the other half stays zero) so the phase C
        # matmuls contract over K=128 at full PE rate.
        uctxA = pp.tile([P, S], F16, name="uctxA")
        uctxB = pp.tile([P, S], F16, name="uctxB")
        uctxh = [uctxA, uctxB]
        nc.vector.memset(uctxA[HD:P, :], 0.0)
        nc.vector.memset(uctxB[0:HD, :], 0.0)
        # per-(head, s-tile) reciprocal softmax denominators as
        # per-partition columns
        rd = pp.tile([P, 2, NT], F32, name="rd")

        # ---------------- Phase A ------------------------------------------
        # All three inputs go through the DMA-transpose XBAR (sync HWDGE
        # queue, the serial resource) in variable-size row-range units,
        # ordered by when phase B first consumes them: k.h0, then just q's
        # first 512-row block (so exp starts ~17us in), k.h1, v (PV is a full
        # iteration behind), then the rest of q.
        SH = S // 2
        xt_tiles = {}  # (which, lo) -> list of 4 xt tiles

        def emit_transposes(which, lo, nrows):
            src = {"k": xk, "v": xv, "q": xq}[which]
            xts = []
            for dt_ in range(4):
                xt = xtp.tile([P, SH], F16, tag="xt", name="xt")[:, :nrows]
                nc.sync.dma_start(
                    xt,
                    src[lo : lo + nrows, dt_ * P : (dt_ + 1) * P],
                    transpose=True,
                )
                xts.append(xt)
            xt_tiles[(which, lo)] = xts

        def emit_proj(which, lo, nrows):
            w = {"k": wks, "v": wvs, "q": wqs}[which]
            xts = xt_tiles[(which, lo)]
            for sbl in range(nrows // 512):
                sb = lo // 512 + sbl
                cols = slice(sb * 512, (sb + 1) * 512)
                lcol = slice(sbl * 512, (sbl + 1) * 512)
                acc = mp.tile([P, 512], F32, tag="mA", name="acc")
                for dt_ in range(4):
                    nc.tensor.matmul(
                        acc,
                        lhsT=w[:, dt_, :],
                        rhs=xts[dt_][:, lcol],
                        start=(dt_ == 0),
                        stop=(dt_ == 3),
                    )
                if which == "q":
                    nc.vector.tensor_scalar_add(
                        qT0[0:HD, cols], acc[0:HD, :], bqs[0:HD, :]
                    )
                    nc.vector.tensor_scalar_add(
                        qT1[HD:P, cols], acc[HD:P, :], bqs[HD:P, :]
                    )
                elif which == "k":
                    nc.vector.tensor_scalar_add(kT[:, cols], acc[:], bks[:])
                else:
                    vt = vsp.tile([P, 512], F16, tag="vt", name="vt")
                    nc.vector.tensor_scalar_add(vt, acc[:], bvs[:])
                    for j in range(4):
                        kt_i = sb * 4 + j
                        ps2 = mp.tile([P, P], F16, tag="mA", name="ps2")
                        nc.tensor.transpose(
                            ps2, vt[:, j * P : (j + 1) * P], ident16
                        )
                        nc.vector.tensor_copy(
                            out=vaug[:, 0, kt_i, 0:HD], in_=ps2[:, 0:HD]
                        )
                        nc.vector.tensor_copy(
                            out=vaug[:, 1, kt_i, HD:P], in_=ps2[:, HD:P]
                        )

        # ------- Phase B: attention, PV lagged one iteration ----------------
        iters = [(qb, h) for qb in range(QB) for h in (0, 1)]
        CPI = (NT + CH - 1) // CH  # chunks per iteration
        chunks = [
            (it_idx, c0)
            for it_idx in range(len(iters))
            for c0 in range(0, NT, CH)
        ]
        LAG = CPI  # PV trails QK/exp by one full iteration
        lg_tiles = {}
        pt_tiles = {}
        pv_tiles = {}

        def emit_qk(j):
            it_idx, c0 = chunks[j]
            qb, h = iters[it_idx]
            qcols = slice(qb * 512, (qb + 1) * 512)
            n = min(CH, NT - c0)
            lg = lgp.tile([P, CH * 512], F32, tag="lg", name="lg")
            for i in range(n):
                kt_i = c0 + i
                nc.tensor.matmul(
                    lg[:, i * 512 : (i + 1) * 512],
                    lhsT=kT[:, kt_i * P : (kt_i + 1) * P],
                    rhs=qTh[h][:, qcols],
                    start=True,
                    stop=True,
                )
            lg_tiles[j] = lg

        def emit_exp(j):
            it_idx, c0 = chunks[j]
            n = min(CH, NT - c0)
            lg = lg_tiles.pop(j)
            ptt = ptp.tile([P, CH * 512], F16, tag="pt", name="ptt")
            nc.scalar.activation(
                ptt[:, : n * 512], lg[:, : n * 512], EXP, scale=0.125
            )
            pt_tiles[j] = ptt

        def emit_pv(j):
            it_idx, c0 = chunks[j]
            qb, h = iters[it_idx]
            n = min(CH, NT - c0)
            ptt = pt_tiles.pop(j)
            if it_idx not in pv_tiles:
                pv_tiles[it_idx] = pbp.tile(
                    [P, 512], F32, tag="pb", name="pv_acc"
                )
            pv_acc = pv_tiles[it_idx]
            for i in range(n):
                kt_i = c0 + i
                nc.tensor.matmul(
                    pv_acc,
                    lhsT=vaug[:, h, kt_i, :],
                    rhs=ptt[:, i * 512 : (i + 1) * 512],
                    start=(kt_i == 0),
                    stop=(kt_i == NT - 1),
                )
            if c0 + CH >= NT:
                finish_iter(it_idx)

        def finish_iter(it_idx):
            # [uctx.T | denom] -> SBUF; denominator row -> per-partition
            # columns via PE transposes; [128,1] column reciprocals (cheap
            # multi-partition DVE).  Normalization happens in phase C.
            qb, h = iters[it_idx]
            qcols = slice(qb * 512, (qb + 1) * 512)
            rows = slice(h * HD, (h + 1) * HD)
            pv_acc = pv_tiles.pop(it_idx)
            dr = HD if h == 0 else 0  # denominator row in the PV output
            # fp16 staging: halves the copy, and the tail transposes run at
            # 1 cycle/row instead of fp32's 2
            uc = ucp.tile([P, 512], F16, tag="uc", name="uc")
            nc.vector.tensor_copy(out=uc, in_=pv_acc[:])
            nc.vector.tensor_copy(out=uctxh[h][rows, qcols], in_=uc[rows, :])
            for sl in range(4):
                st = 4 * qb + sl
                tps = mp.tile([P, P], F16, tag="mA", name="tps")
                nc.tensor.transpose(
                    tps, uc[:, sl * P : (sl + 1) * P], ident16
                )
                nc.vector.reciprocal(
                    rd[:, h, st : st + 1], tps[:, dr : dr + 1]
                )

        # phase C in two half-steps per s-tile (single mp PSUM bank, no
        # PE-waits-on-DVE): A: ops0 = uctxA.T @ Wo (K=128, h0 rows live);
        # ob0 = ops0 * rd0.  B: ops1 = uctxB.T @ Wo; ob = ops1*rd1 + ob0; DMA.
        ob0_tiles = {}

        def emit_phase_c_half(st, phase):
            stcols = slice(st * P, (st + 1) * P)
            if phase == 0:
                ops0 = mp.tile([P, D], F32, tag="mA", name="ops0")
                nc.tensor.matmul(
                    ops0, lhsT=uctxA[:, stcols], rhs=wos, start=True, stop=True
                )
                ob0 = obp.tile([P, D], F32, tag="ob0", name="ob0")
                nc.vector.tensor_scalar_mul(ob0, ops0[:], rd[:, 0, st : st + 1])
                ob0_tiles[st] = ob0
            else:
                ops1 = mp.tile([P, D], F32, tag="mA", name="ops1")
                nc.tensor.matmul(
                    ops1, lhsT=uctxB[:, stcols], rhs=wos, start=True, stop=True
                )
                ob0 = ob0_tiles.pop(st)
                ob = obp.tile([P, D], F16, tag="ob", name="ob")
                nc.vector.scalar_tensor_tensor(
                    out=ob,
                    in0=ops1[:],
                    scalar=rd[:, 1, st : st + 1],
                    in1=ob0[:],
                    op0=mybir.AluOpType.mult,
                    op1=mybir.AluOpType.add,
                )
                nc.sync.dma_start(out[st * P : (st + 1) * P, :], ob)

        # ---- emission schedule ----
        # XBAR units (sync queue, serial), in first-consumption order;
        # projections emitted in the same order so the single mp PSUM bank's
        # rotation matches execution.
        units = (
            ("k", 0, SH),
            ("q", 0, 512),
            ("k", SH, SH),
            ("v", 0, SH),
            ("v", SH, SH),
            ("q", 512, 512),
            ("q", 1024, 1024),
            ("q", 2048, 2048),
        )
        for u in units:
            emit_transposes(*u)
        for u in units:
            emit_proj(*u)

        n_chunks = len(chunks)
        pc_queue = []  # pending phase C half-steps, drained 1 per loop step
        emit_qk(0)
        for j in range(n_chunks + LAG):
            if j < n_chunks:
                emit_exp(j)
                if j + 1 < n_chunks:
                    emit_qk(j + 1)
            if j >= LAG:
                emit_pv(j - LAG)
                it_idx, c0 = chunks[j - LAG]
                if c0 + CH >= NT and it_idx % 2 == 1:
                    qb = it_idx // 2
                    for sl in range(4):
                        pc_queue.append((4 * qb + sl, 0))
                        pc_queue.append((4 * qb + sl, 1))
            if pc_queue:
                emit_phase_c_half(*pc_queue.pop(0))
        while pc_queue:
            emit_phase_c_half(*pc_queue.pop(0))


def build(S=S_FULL, enable_asserts=False):
    nc = bacc.Bacc(
        "TRN2",
        target_bir_lowering=False,
        debug=False,
        enable_asserts=enable_asserts,
        num_devices=N_CORES,
    )
    xq = nc.dram_tensor("xq", [S, D], F16, kind="ExternalInput").ap()
    xk = nc.dram_tensor("xk", [S, D], F16, kind="ExternalInput").ap()
    xv = nc.dram_tensor("xv", [S, D], F16, kind="ExternalInput").ap()
    # packed [Wk; Wq; Wv] and [bk | bq | bv] (fewer pre-transpose DMAs)
    wq = nc.dram_tensor("wpack", [3 * D, GD], F16, kind="ExternalInput").ap()
    wo = nc.dram_tensor("wo", [GD, D], F16, kind="ExternalInput").ap()
    bq = nc.dram_tensor("bpack", [3 * GD], F32, kind="ExternalInput").ap()
    out = nc.dram_tensor("out", [S, D], F16, kind="ExternalOutput").ap()
    io = (xq, xk, xv, wq, None, None, wo, bq, None, None, None, out)
    with tile.TileContext(nc) as tc:
        _emit(tc, S, io)
    nc.compile()
    return nc


def make_in_maps(queries, keys, values, Wq, bq, Wk, bk, Wv, bv, Wo, bo):
    f16 = lambda a: np.ascontiguousarray(np.asarray(a, dtype=np.float32).astype(np.float16))
    f32 = lambda a: np.ascontiguousarray(np.asarray(a, dtype=np.float32))
    in_maps = []
    for c in range(N_CORES):
        b, g = divmod(c, 4)
        sl = slice(g * GD, (g + 1) * GD)
        in_maps.append(
            {
                "xq": f16(queries[b]),
                "xk": f16(keys[b]),
                "xv": f16(values[b]),
                "wpack": f16(
                    np.concatenate(
                        [
                            np.asarray(Wk)[:, sl],
                            np.asarray(Wq)[:, sl],
                            np.asarray(Wv)[:, sl],
                        ],
                        axis=0,
                    )
                ),
                "wo": f16(np.asarray(Wo)[sl, :]),
                "bpack": f32(
                    np.concatenate(
                        [
                            np.asarray(bk)[sl],
                            np.asarray(bq)[sl],
                            np.asarray(bv)[sl],
                        ]
                    )
                ),
            }
        )
    return in_maps


_NC = None
last_results = None


def kernel(queries, keys, values, Wq, bq, Wk, bk, Wv, bv, Wo, bo):
    global _NC, last_results
    if _NC is None:
        _NC = build(S_FULL)
    in_maps = make_in_maps(
        queries, keys, values, Wq, bq, Wk, bk, Wv, bv, Wo, bo
    )
    res = run_bass_kernel_spmd(
        _NC,
        in_maps,
        core_ids=list(range(N_CORES)),
        trace=bool(int(os.environ.get("MHA_TRACE", "0"))),
    )
    last_results = res
    outs = [np.asarray(res.results[c]["out"], dtype=np.float32) for c in range(N_CORES)]
    full = np.empty((B_FULL, S_FULL, D), dtype=np.float32)
    bo32 = np.asarray(bo, dtype=np.float32)
    for b in range(B_FULL):
        full[b] = outs[4 * b] + outs[4 * b + 1] + outs[4 * b + 2] + outs[4 * b + 3]
        full[b] += bo32
    return full


# revision 65
# speedup vs baseline: 1.0304x; 1.0179x over previous
"""Multi-head attention (B=2, S=4096, D=512, H=8) on 8 Trainium2 NeuronCores.

Sharding: core c handles batch b = c // 4 and head-group g = c % 4 (2 heads =
columns/rows [128g : 128g+128] of the projection weights).  Each core runs its
2 heads' attention over the full sequence plus the partial output projection
through the matching 128 rows of Wo; the host sums the 4 partials per batch
and adds bo (pure unshard for row-parallel Wo).

Numerics: fp16 storage for X/W/q/k/v/P/ctx, fp32 PSUM accumulation, fp32
softmax denominators, fp16 output partials (summed fp32 host-side).

v2 pipeline (vs baseline):
  - DMA-transposes ordered k.h0, q.h0, k.h1, v.h0, v.h1, q.h1 so QK/exp can
    start ~25us in; k is fully resident right when iter 0 needs its 2nd half.
  - PV runs one full iteration behind QK/exp (P tiles buffered in SBUF), so
    early PVs never block the PE queue waiting on v transposes.
  - softmax normalization via rank-1 broadcast: the denominator row of
    [uctx.T | denom] is reciprocated as a row (DVE), broadcast to a [128,512]
    PSUM tile with a 1-row fp16 matmul, and multiplied into uctx during the
    fp16 downcast (DVE).  No PE tail transposes, no ACT copies.
  - output projection is ONE matmul per s-tile (both heads contract together
    since uctx16 rows 0:64 / 64:128 are the two heads), fp16 out partials.
"""

import os

import numpy as np

import concourse.bass as bass
import concourse.tile as tile
from concourse import bacc, mybir
from concourse.bass_utils import run_bass_kernel_spmd
from concourse.masks import make_identity

P = 128
D = 512
GD = 128  # head-group width: 2 heads x 64
HD = 64
S_FULL = 4096
B_FULL = 2
N_CORES = 8
F32 = mybir.dt.float32
F16 = mybir.dt.float16
EXP = mybir.ActivationFunctionType.Exp


def _emit(tc, S, io):
    nc = tc.nc
    NT = S // P  # 128-wide s/k tiles
    SB = S // 512  # 512-wide s blocks
    QB = S // 512  # query blocks
    CH = 3  # key-tiles per exp chunk (3 PSUM banks, x2 buffered)

    xq, xk, xv, wq, wk, wv, wo, bq, bk, bv, bo, out = io

    with (
        tc.tile_pool(name="persist", bufs=1) as pp,
        tc.tile_pool(name="lgp", bufs=2, space="PSUM") as lgp,
        tc.tile_pool(name="mpsum", bufs=1, space="PSUM") as mp,
        tc.tile_pool(name="pbp", bufs=1, space="PSUM") as pbp,
        tc.tile_pool(name="xtp", bufs=16) as xtp,
        tc.tile_pool(name="vstage", bufs=4) as vsp,
        tc.tile_pool(name="ptp", bufs=14) as ptp,
        tc.tile_pool(name="ucp", bufs=3) as ucp,
        tc.tile_pool(name="obp", bufs=4) as obp,
    ):
        ident16 = pp.tile([P, P], F16, name="ident16")
        make_identity(nc, ident16)

        # fp16 weights (pre-cast AND pre-packed on host) — on the sync queue
        # BEFORE the DMA-transposes: concurrent non-transpose DMA traffic
        # interleaves exclusively with the XBAR and opens ~8.5us gaps between
        # transposes, so fewer DMAs here = earlier transpose start.
        # wq holds [Wk; Wq; Wv] stacked (k first: its projection is needed
        # soonest); bq holds [bk | bq | bv] columns.
        wall = pp.tile([P, 12, GD], F16, name="wall")
        nc.sync.dma_start(wall, wq.rearrange("(w t p) m -> p (w t) m", p=P, t=4))
        wks = wall[:, 0:4, :]
        wqs = wall[:, 4:8, :]
        wvs = wall[:, 8:12, :]
        wos = pp.tile([P, D], F16, name="wos")
        nc.sync.dma_start(wos, wo)
        ball = pp.tile([P, 3], F32, name="ball")
        nc.sync.dma_start(ball, bq.rearrange("(w p) -> p w", p=P))
        bks = ball[:, 0:1]
        bqs = ball[:, 1:2]
        bvs = ball[:, 2:3]

        # big persistent activations (all fp16).  Partition-offset memsets
        # stay on the DVE (proven on HW); gpsimd only does full-partition
        # memsets (no DMA-fabric use either way).
        kT = pp.tile([P, S], F16, name="kT")
        # q per-head, zero-padded to 128 partitions so QK contracts over
        # K=128 (K=64 matmuls stream at ~half rate on the PE)
        qT0 = pp.tile([P, S], F16, name="qT0")
        qT1 = pp.tile([P, S], F16, name="qT1")
        qTh = [qT0, qT1]
        nc.vector.memset(qT0[HD:P, :], 0.0)
        nc.vector.memset(qT1[0:HD, :], 0.0)
        # v in natural [keys, hd] layout per k-tile, per head; head0's v at
        # free cols 0:64 with a ones-column at 64, head1's v at cols 64:128
        # with its ones-column at 0 — so PV output rows line up with the
        # head's rows in uctx16 and the denominator row is 64 / 0.
        vaug = pp.tile([P, 2, NT, P], F16, name="vaug")
        nc.gpsimd.memset(vaug, 0.0)
        nc.gpsimd.memset(vaug[:, 0, :, HD : HD + 1], 1.0)
        nc.gpsimd.memset(vaug[:, 1, :, 0:1], 1.0)
        # unnormalized ctx.T, one zero-padded tile per head (head h's rows
        # live at 64h:64h+64, the other half stays zero) so the phase C
        # matmuls contract over K=128 at full PE rate.
        uctxA = pp.tile([P, S], F16, name="uctxA")
        uctxB = pp.tile([P, S], F16, name="uctxB")
        uctxh = [uctxA, uctxB]
        nc.vector.memset(uctxA[HD:P, :], 0.0)
        nc.vector.memset(uctxB[0:HD, :], 0.0)
        # per-(head, s-tile) reciprocal softmax denominators as
        # per-partition columns
        rd = pp.tile([P, 2, NT], F32, name="rd")

        # ---------------- Phase A ------------------------------------------
        # All three inputs go through the DMA-transpose XBAR (sync HWDGE
        # queue, the serial resource) in variable-size row-range units,
        # ordered by when phase B first consumes them: k.h0, then just q's
        # first 512-row block (so exp starts ~17us in), k.h1, v (PV is a full
        # iteration behind), then the rest of q.
        SH = S // 2
        xt_tiles = {}  # (which, lo) -> list of 4 xt tiles

        def emit_transposes(which, lo, nrows):
            src = {"k": xk, "v": xv, "q": xq}[which]
            xts = []
            for dt_ in range(4):
                xt = xtp.tile([P, SH], F16, tag="xt", name="xt")[:, :nrows]
                nc.sync.dma_start(
                    xt,
                    src[lo : lo + nrows, dt_ * P : (dt_ + 1) * P],
                    transpose=True,
                )
                xts.append(xt)
            xt_tiles[(which, lo)] = xts

        def emit_proj(which, lo, nrows):
            w = {"k": wks, "v": wvs, "q": wqs}[which]
            xts = xt_tiles[(which, lo)]
            for sbl in range(nrows // 512):
                sb = lo // 512 + sbl
                cols = slice(sb * 512, (sb + 1) * 512)
                lcol = slice(sbl * 512, (sbl + 1) * 512)
                acc = mp.tile([P, 512], F32, tag="mA", name="acc")
                for dt_ in range(4):
                    nc.tensor.matmul(
                        acc,
                        lhsT=w[:, dt_, :],
                        rhs=xts[dt_][:, lcol],
                        start=(dt_ == 0),
                        stop=(dt_ == 3),
                    )
                if which == "q":
                    nc.vector.tensor_scalar_add(
                        qT0[0:HD, cols], acc[0:HD, :], bqs[0:HD, :]
                    )
                    nc.vector.tensor_scalar_add(
                        qT1[HD:P, cols], acc[HD:P, :], bqs[HD:P, :]
                    )
                elif which == "k":
                    nc.vector.tensor_scalar_add(kT[:, cols], acc[:], bks[:])
                else:
                    vt = vsp.tile([P, 512], F16, tag="vt", name="vt")
                    nc.vector.tensor_scalar_add(vt, acc[:], bvs[:])
                    for j in range(4):
                        kt_i = sb * 4 + j
                        ps2 = mp.tile([P, P], F16, tag="mA", name="ps2")
                        nc.tensor.transpose(
                            ps2, vt[:, j * P : (j + 1) * P], ident16
                        )
                        nc.vector.tensor_copy(
                            out=vaug[:, 0, kt_i, 0:HD], in_=ps2[:, 0:HD]
                        )
                        nc.vector.tensor_copy(
                            out=vaug[:, 1, kt_i, HD:P], in_=ps2[:, HD:P]
                        )

        # ------- Phase B: attention, PV lagged one iteration ----------------
        iters = [(qb, h) for qb in range(QB) for h in (0, 1)]
        CPI = (NT + CH - 1) // CH  # chunks per iteration
        chunks = [
            (it_idx, c0)
            for it_idx in range(len(iters))
            for c0 in range(0, NT, CH)
        ]
        LAG = CPI  # PV trails QK/exp by one full iteration
        lg_tiles = {}
        pt_tiles = {}
        pv_tiles = {}

        def emit_qk(j):
            it_idx, c0 = chunks[j]
            qb, h = iters[it_idx]
            qcols = slice(qb * 512, (qb + 1) * 512)
            n = min(CH, NT - c0)
            lg = lgp.tile([P, CH * 512], F32, tag="lg", name="lg")
            for i in range(n):
                kt_i = c0 + i
                nc.tensor.matmul(
                    lg[:, i * 512 : (i + 1) * 512],
                    lhsT=kT[:, kt_i * P : (kt_i + 1) * P],
                    rhs=qTh[h][:, qcols],
                    start=True,
                    stop=True,
                )
            lg_tiles[j] = lg

        def emit_exp(j):
            it_idx, c0 = chunks[j]
            n = min(CH, NT - c0)
            lg = lg_tiles.pop(j)
            ptt = ptp.tile([P, CH * 512], F16, tag="pt", name="ptt")
            nc.scalar.activation(
                ptt[:, : n * 512], lg[:, : n * 512], EXP, scale=0.125
            )
            pt_tiles[j] = ptt

        def emit_pv(j):
            it_idx, c0 = chunks[j]
            qb, h = iters[it_idx]
            n = min(CH, NT - c0)
            ptt = pt_tiles.pop(j)
            if it_idx not in pv_tiles:
                pv_tiles[it_idx] = pbp.tile(
                    [P, 512], F32, tag="pb", name="pv_acc"
                )
            pv_acc = pv_tiles[it_idx]
            for i in range(n):
                kt_i = c0 + i
                nc.tensor.matmul(
                    pv_acc,
                    lhsT=vaug[:, h, kt_i, :],
                    rhs=ptt[:, i * 512 : (i + 1) * 512],
                    start=(kt_i == 0),
                    stop=(kt_i == NT - 1),
                )
            if c0 + CH >= NT:
                finish_iter(it_idx)

        def finish_iter(it_idx):
            # [uctx.T | denom] -> SBUF; denominator row -> per-partition
            # columns via PE transposes; [128,1] column reciprocals (cheap
            # multi-partition DVE).  Normalization happens in phase C.
            qb, h = iters[it_idx]
            qcols = slice(qb * 512, (qb + 1) * 512)
            rows = slice(h * HD, (h + 1) * HD)
            pv_acc = pv_tiles.pop(it_idx)
            dr = HD if h == 0 else 0  # denominator row in the PV output
            # fp16 staging: halves the copy, and the tail transposes run at
            # 1 cycle/row instead of fp32's 2
            uc = ucp.tile([P, 512], F16, tag="uc", name="uc")
            nc.vector.tensor_copy(out=uc, in_=pv_acc[:])
            nc.vector.tensor_copy(out=uctxh[h][rows, qcols], in_=uc[rows, :])
            for sl in range(4):
                st = 4 * qb + sl
                tps = mp.tile([P, P], F16, tag="mA", name="tps")
                nc.tensor.transpose(
                    tps, uc[:, sl * P : (sl + 1) * P], ident16
                )
                nc.vector.reciprocal(
                    rd[:, h, st : st + 1], tps[:, dr : dr + 1]
                )

        # phase C in two half-steps per s-tile (single mp PSUM bank, no
        # PE-waits-on-DVE): A: ops0 = uctxA.T @ Wo (K=128, h0 rows live);
        # ob0 = ops0 * rd0.  B: ops1 = uctxB.T @ Wo; ob = ops1*rd1 + ob0; DMA.
        ob0_tiles = {}

        def emit_phase_c_half(st, phase):
            stcols = slice(st * P, (st + 1) * P)
            if phase == 0:
                ops0 = mp.tile([P, D], F32, tag="mA", name="ops0")
                nc.tensor.matmul(
                    ops0, lhsT=uctxA[:, stcols], rhs=wos, start=True, stop=True
                )
                ob0 = obp.tile([P, D], F32, tag="ob0", name="ob0")
                nc.vector.tensor_scalar_mul(ob0, ops0[:], rd[:, 0, st : st + 1])
                ob0_tiles[st] = ob0
            else:
                ops1 = mp.tile([P, D], F32, tag="mA", name="ops1")
                nc.tensor.matmul(
                    ops1, lhsT=uctxB[:, stcols], rhs=wos, start=True, stop=True
                )
                ob0 = ob0_tiles.pop(st)
                ob = obp.tile([P, D], F16, tag="ob", name="ob")
                nc.vector.scalar_tensor_tensor(
                    out=ob,
                    in0=ops1[:],
                    scalar=rd[:, 1, st : st + 1],
                    in1=ob0[:],
                    op0=mybir.AluOpType.mult,
                    op1=mybir.AluOpType.add,
                )
                # split output DMAs across two queues so the end-drain's
                # DMA issues don't serialize on sync alone; gpsimd traffic
                # is safe here (the XBAR transposes finished long before,
                # so no DMA-fabric interleave penalty).  s-tiles of the
                # first query block stay on sync (they overlap the tail of
                # the transpose stream).
                if st >= 8 and st % 2 == 1:
                    nc.gpsimd.dma_start(out[st * P : (st + 1) * P, :], ob)
                else:
                    nc.sync.dma_start(out[st * P : (st + 1) * P, :], ob)

        # ---- emission schedule ----
        # XBAR units (sync queue, serial), in first-consumption order;
        # projections emitted in the same order so the single mp PSUM bank's
        # rotation matches execution.
        units = (
            ("k", 0, SH),
            ("q", 0, 512),
            ("k", SH, SH),
            ("v", 0, SH),
            ("v", SH, SH),
            ("q", 512, 512),
            ("q", 1024, 1024),
            ("q", 2048, 2048),
        )
        for u in units:
            emit_transposes(*u)
        for u in units:
            emit_proj(*u)

        n_chunks = len(chunks)
        pc_queue = []  # pending phase C half-steps, drained 1 per loop step
        emit_qk(0)
        for j in range(n_chunks + LAG):
            if j < n_chunks:
                emit_exp(j)
                if j + 1 < n_chunks:
                    emit_qk(j + 1)
            if j >= LAG:
                emit_pv(j - LAG)
                it_idx, c0 = chunks[j - LAG]
                if c0 + CH >= NT and it_idx % 2 == 1:
                    qb = it_idx // 2
                    for sl in range(4):
                        pc_queue.append((4 * qb + sl, 0))
                        pc_queue.append((4 * qb + sl, 1))
            if pc_queue:
                emit_phase_c_half(*pc_queue.pop(0))
        while pc_queue:
            emit_phase_c_half(*pc_queue.pop(0))


def build(S=S_FULL, enable_asserts=False):
    nc = bacc.Bacc(
        "TRN2",
        target_bir_lowering=False,
        debug=False,
        enable_asserts=enable_asserts,
        num_devices=N_CORES,
    )
    xq = nc.dram_tensor("xq", [S, D], F16, kind="ExternalInput").ap()
    xk = nc.dram_tensor("xk", [S, D], F16, kind="ExternalInput").ap()
    xv = nc.dram_tensor("xv", [S, D], F16, kind="ExternalInput").ap()
    # packed [Wk; Wq; Wv] and [bk | bq | bv] (fewer pre-transpose DMAs)
    wq = nc.dram_tensor("wpack", [3 * D, GD], F16, kind="ExternalInput").ap()
    wo = nc.dram_tensor("wo", [GD, D], F16, kind="ExternalInput").ap()
    bq = nc.dram_tensor("bpack", [3 * GD], F32, kind="ExternalInput").ap()
    out = nc.dram_tensor("out", [S, D], F16, kind="ExternalOutput").ap()
    io = (xq, xk, xv, wq, None, None, wo, bq, None, None, None, out)
    with tile.TileContext(nc) as tc:
        _emit(tc, S, io)
    nc.compile()
    return nc


def make_in_maps(queries, keys, values, Wq, bq, Wk, bk, Wv, bv, Wo, bo):
    f16 = lambda a: np.ascontiguousarray(np.asarray(a, dtype=np.float32).astype(np.float16))
    f32 = lambda a: np.ascontiguousarray(np.asarray(a, dtype=np.float32))
    in_maps = []
    for c in range(N_CORES):
        b, g = divmod(c, 4)
        sl = slice(g * GD, (g + 1) * GD)
        in_maps.append(
            {
                "xq": f16(queries[b]),
                "xk": f16(keys[b]),
                "xv": f16(values[b]),
                "wpack": f16(
                    np.concatenate(
                        [
                            np.asarray(Wk)[:, sl],
                            np.asarray(Wq)[:, sl],
                            np.asarray(Wv)[:, sl],
                        ],
                        axis=0,
                    )
                ),
                "wo": f16(np.asarray(Wo)[sl, :]),
                "bpack": f32(
                    np.concatenate(
                        [
                            np.asarray(bk)[sl],
                            np.asarray(bq)[sl],
                            np.asarray(bv)[sl],
                        ]
                    )
                ),
            }
        )
    return in_maps


_NC = None
last_results = None


def kernel(queries, keys, values, Wq, bq, Wk, bk, Wv, bv, Wo, bo):
    global _NC, last_results
    if _NC is None:
        _NC = build(S_FULL)
    in_maps = make_in_maps(
        queries, keys, values, Wq, bq, Wk, bk, Wv, bv, Wo, bo
    )
    res = run_bass_kernel_spmd(
        _NC,
        in_maps,
        core_ids=list(range(N_CORES)),
        trace=bool(int(os.environ.get("MHA_TRACE", "0"))),
    )
    last_results = res
    outs = [np.asarray(res.results[c]["out"], dtype=np.float32) for c in range(N_CORES)]
    full = np.empty((B_FULL, S_FULL, D), dtype=np.float32)
    bo32 = np.asarray(bo, dtype=np.float32)
    for b in range(B_FULL):
        full[b] = outs[4 * b] + outs[4 * b + 1] + outs[4 * b + 2] + outs[4 * b + 3]
        full[b] += bo32
    return full
